# revision 46
# baseline (speedup 1.0000x reference)
"""Trainium2 Bass kernel for nn_GCLSTMModel_48868137894020 (v8).

Algebraic reduction (H0 = C0 = 0 kills the cheb convs, the forget gate,
and peep[0/1]); per layer (d = 140 then 280), X = input:
  I = sigmoid(X @ W[0] + cb[0] + b[0])
  T = tanh   (X @ W[2] + cb[2] + b[2])
  C = I * T
  O = sigmoid(X @ W[3] + cb[3] + b[3] + peep[2] * C)
  X' = relu(O * tanh(C)) == max(tanh(C), 0) * O
then out = relu(X'' @ fc_w + fc_b).

Layout: everything transposed (features on partitions, nodes on the
free dim), bf16 matmuls with fp32 PSUM, biases folded via a ones row.

v8 = the v6 compute structure on the v3 two-ring DMA plan (each part
measured best in isolation via neuron-profile traces):
  - sigmoid(x) = (1 + tanh(x/2)) / 2: I-gate weights halved on the
    host, so ONE tanh ACTIVATE per chunk covers gates I and T together
    (ACT-engine serialization was the tail bottleneck).  C' := 2C =
    (1 + tI) * tT is one DVE STT per chunk; tanh(C) = tanh(0.5 * C')
    uses ACT's free scale; peephole scalars are halved on the host.
  - Tile tracks PSUM dependencies at whole-tile granularity, so every
    matmul REGION gets its own psum tile (p1a/p1b/p1o0/... ) - readers
    then wait only for their own producers, and the scheduler runs all
    nine k0 matmuls of the L2 stream before the k1s arrive.
  - DMA: wpk + small w2k1 on the sync HWDGE ring, the big wb pack on
    the scalar ring (see the inline comment for the straggler/table
    trade-offs that pinned this assignment).
  - A warm-up SIGMOID heads the activation sequence: the act-table
    pass keys its table set off the first activation function, and the
    sigmoid set also contains tanh - leading with tanh loads a
    tanh-only set and re-loads mid-chain (v4, +1.3us).
  - Output DMA on the warm sync ring after the tile context; its
    flight hides under the compiler's fixed sem-reset epilogue.

Sharding: problem is tiny (N=35); all 8 cores run the identical program
on replicated inputs (no collectives), output taken from core 0.
"""

import sys

for _p in ("/opt/trn_rl_repo", "/opt/pypackages"):
    if _p not in sys.path:
        sys.path.append(_p)

from contextlib import ExitStack

import numpy as np
import ml_dtypes

import concourse.bacc as bacc
import concourse.bass as bass
import concourse.mybir as mybir
import concourse.tile as tile

F32 = mybir.dt.float32
BF16 = mybir.dt.bfloat16
AF = mybir.ActivationFunctionType
OP = mybir.AluOpType
GATES = (0, 2, 3)  # I, T (cell), O - forget gate (1) is dead
N = 35
D1 = 140
D2 = 280
N_CORES = 8
BF = ml_dtypes.bfloat16

# L1 feature chunks (140 = 128 + 12), L2 chunks (280 = 128 + 256 + 24).
C1 = ((0, 128), (128, 140))
C2 = ((0, 128), (128, 256), (256, 280))


def build_nc() -> bass.Bass:
    nc = bacc.Bacc()

    # wpk: [128, 466] bf16 - adj + ALL layer-1 weights + peepholes:
    #   rows 0:36: cols 0:36 = [adj; ones], 36:176 = [W1[I]/2; bias/2],
    #   176:316 = [W1[T]; bias], 316:456 = [W1[O]; bias]; cols 456:466
    #   rows 0:128 = halved peep scalars as a bit-cast fp32 [128, 5].
    wpk = nc.dram_tensor("wpk", [36, 456], BF16, kind="ExternalInput")
    # W2 k-chunk1 (input features 128:140 + bias row): [13, 3, 280],
    # gate order (I/2, T, O).
    w2k1 = nc.dram_tensor("w2k1", [13, 3, D2], BF16, kind="ExternalInput")
    # wb: [128, 948] bf16 - layer-2 + FC weights (k-chunk0):
    #   cols 0:280 = W2[I]/2, 280:560 = W2[T], 560:840 = W2[O],
    #   840:948 = [fc_w; fc_b] row-chunks as 3x36.
    wb = nc.dram_tensor("wb", [128, 958], BF16, kind="ExternalInput")
    out = nc.dram_tensor("out", [N, N], F32, kind="ExternalOutput")

    out_sb = nc.alloc_sbuf_tensor("out_sbuf", [N, N], F32)
    out_sem = nc.alloc_semaphore("out_dma_sem")

    with ExitStack() as ctx:
        tc = ctx.enter_context(tile.TileContext(nc))
        sb = ctx.enter_context(tc.tile_pool(name="sb", bufs=1))
        ps1 = ctx.enter_context(tc.tile_pool(name="ps1", bufs=3, space="PSUM"))
        ps2 = ctx.enter_context(tc.tile_pool(name="ps2", bufs=5, space="PSUM"))

        # ---- input DMAs: two HWDGE rings issue in parallel ----
        # wpk + small w2k1 on the sync ring, the big wb pack on the
        # scalar ring: queueing wb behind wpk on ONE ring delays wpk's
        # final completion increment by 1-2us (straggler SDMA engine,
        # v5/v6 traces).  The scalar-ring DMA costs a spurious second
        # act-table load, but both loads finish before the first gate
        # activation needs the table (v3/v3d traces).
        wpk_sb = sb.tile([36, 456], BF16, tag="wpk")
        nc.sync.dma_start(out=wpk_sb, in_=wpk[:, :])
        wb_sb = sb.tile([128, 958], BF16, tag="wb")
        nc.scalar.dma_start(out=wb_sb, in_=wb[:, :])
        w2k1_sb = sb.tile([13, 3, D2], BF16, tag="w2k1")
        nc.sync.dma_start(out=w2k1_sb, in_=w2k1[:, :, :])

        adjp = wpk_sb[0:36, 0:36]
        aux_v = wb_sb[:, 948:958].bitcast(F32)  # [128, 5] peep/2 scalars

        # warm-up sigmoid (no DMA deps): keeps the sigmoid table set
        # (which covers tanh) as the one the table loads converge on,
        # in the DMA shadow.
        warm_src = sb.tile([1, 1], F32, tag="warm_src")
        nc.vector.memset(warm_src[:, :], 0.25)
        warm = sb.tile([1, 1], F32, tag="warm")
        nc.scalar.activation(warm[0:1, 0:1], warm_src[0:1, 0:1], AF.Sigmoid)

        # ones rows for the bias folds + zero-fill for the garbage rows
        # the merged whole-width tanhs read.
        x1T = sb.tile([128, 72], BF16, tag="x1T")
        nc.vector.memset(x1T[0:13, 36:72], 1.0)
        x2T = sb.tile([128, 108], BF16, tag="x2T")
        nc.vector.memset(x2T[0:25, 72:108], 1.0)
        cp1 = sb.tile([128, 72], BF16, tag="cp1")    # C' = 2C, layer 1
        nc.vector.memset(cp1[0:128, 36:72], 0.0)
        cp2 = sb.tile([128, 108], BF16, tag="cp2")   # C' = 2C, layer 2
        nc.vector.memset(cp2[0:128, 72:108], 0.0)

        # ---- layer 1: psum banks grouped for fine-grained waits ----
        # p1a = [Ic0|Tc0]; p1b = [Ic1|Tc1|Oc1] (12 rows); p1o0 = Oc0.
        p1a = ps1.tile([128, 72], F32, tag="ps1", name="p1a")
        p1b = ps1.tile([12, 108], F32, tag="ps1", name="p1b")
        p1o0 = ps1.tile([128, 36], F32, tag="ps1", name="p1o0")
        l1_lhs = (
            lambda a, b: wpk_sb[0:36, 36 + a : 36 + b],   # I/2
            lambda a, b: wpk_sb[0:36, 176 + a : 176 + b],  # T
            lambda a, b: wpk_sb[0:36, 316 + a : 316 + b],  # O
        )
        l1_dsts = (  # (chunk, gate) -> psum region
            (p1a[0:128, 0:36], p1a[0:128, 36:72], p1o0[0:128, 0:36]),
            (p1b[0:12, 0:36], p1b[0:12, 36:72], p1b[0:12, 72:108]),
        )
        for ci, (a, b) in enumerate(C1):
            for g in (0, 1, 2):
                nc.tensor.matmul(
                    l1_dsts[ci][g],
                    lhsT=l1_lhs[g](a, b), rhs=adjp,
                    start=True, stop=True,
                )
        # ACT: t = tanh([aI/2 | aT]) per chunk -> go1 c0 -> tanh(C) -> go1 c1
        t1 = sb.tile([128, 144], BF16, tag="t1")
        nc.scalar.activation(t1[0:128, 0:72], p1a[:, :], AF.Tanh)
        nc.scalar.activation(t1[0:12, 72:144], p1b[0:12, 0:72], AF.Tanh)
        # DVE per chunk: C' = (1+tI)*tT then po = (p/2)*C' + aO.  The
        # chunk-0 x1T STT is emitted BEFORE the chunk-1 peephole STT:
        # the nine k0 matmuls of the L2 stream need only x1T chunk 0,
        # and the v8 trace showed x1Tc0 head-blocked behind po1c1 on
        # the DVE queue for ~0.25us.
        po1 = sb.tile([128, 72], BF16, tag="po1")
        p1o_regions = (p1o0[0:128, 0:36], p1b[0:12, 72:108])
        nc.vector.scalar_tensor_tensor(
            cp1[0:128, 0:36], in0=t1[0:128, 0:36],
            scalar=1.0, in1=t1[0:128, 36:72],
            op0=OP.add, op1=OP.mult,
        )
        nc.vector.scalar_tensor_tensor(
            po1[0:128, 0:36], in0=cp1[0:128, 0:36],
            scalar=aux_v[0:128, 0:1], in1=p1o_regions[0],
            op0=OP.mult, op1=OP.add,
        )
        nc.vector.scalar_tensor_tensor(
            cp1[0:12, 36:72], in0=t1[0:12, 72:108],
            scalar=1.0, in1=t1[0:12, 108:144],
            op0=OP.add, op1=OP.mult,
        )
        go1 = sb.tile([128, 72], BF16, tag="go1")
        nc.scalar.activation(go1[0:128, 0:36], po1[0:128, 0:36], AF.Sigmoid)
        tc1 = sb.tile([128, 72], BF16, tag="tc1")
        nc.scalar.activation(tc1, cp1[:, :], AF.Tanh, scale=0.5)
        # x1T c0 fires as soon as tc1 + go1 c0 land; the chunk-1 chain
        # (po1c1 -> go1c1 -> x1Tc1) trails it on the DVE/ACT queues.
        nc.vector.scalar_tensor_tensor(
            x1T[0:128, 0:36], in0=tc1[0:128, 0:36], scalar=0.0,
            in1=go1[0:128, 0:36], op0=OP.max, op1=OP.mult,
        )
        nc.vector.scalar_tensor_tensor(
            po1[0:12, 36:72], in0=cp1[0:12, 36:72],
            scalar=aux_v[0:12, 1:2], in1=p1o_regions[1],
            op0=OP.mult, op1=OP.add,
        )
        nc.scalar.activation(go1[0:12, 36:72], po1[0:12, 36:72], AF.Sigmoid)
        nc.vector.scalar_tensor_tensor(
            x1T[0:12, 36:72], in0=tc1[0:12, 36:72], scalar=0.0,
            in1=go1[0:12, 36:72], op0=OP.max, op1=OP.mult,
        )

        # ---- layer 2: psum banks grouped for fine-grained waits ----
        # p2a/p2b/p2c = [I|T] per chunk; p2o01 = [Oc0|Oc1]; p2o2 = Oc2.
        p2it = (
            ps2.tile([128, 72], F32, tag="ps2", name="p2a"),
            ps2.tile([128, 72], F32, tag="ps2", name="p2b"),
            ps2.tile([24, 72], F32, tag="ps2", name="p2c"),
        )
        p2o01 = ps2.tile([128, 72], F32, tag="ps2", name="p2o01")
        p2o2 = ps2.tile([24, 36], F32, tag="ps2", name="p2o2")
        p2o_regions = (
            p2o01[0:128, 0:36], p2o01[0:128, 36:72], p2o2[0:24, 0:36]
        )
        w2k0 = (wb_sb[:, 0:280], wb_sb[:, 280:560], wb_sb[:, 560:840])
        for ci, (a, b) in enumerate(C2):
            cs = b - a
            for g, dst in (
                (0, p2it[ci][0:cs, 0:36]),
                (1, p2it[ci][0:cs, 36:72]),
                (2, p2o_regions[ci]),
            ):
                nc.tensor.matmul(
                    dst,
                    lhsT=w2k0[g][:, a:b], rhs=x1T[0:128, 0:36],
                    start=True, stop=False,
                )
                nc.tensor.matmul(
                    dst,
                    lhsT=w2k1_sb[0:13, g, a:b], rhs=x1T[0:13, 36:72],
                    start=False, stop=True,
                )
        t2 = sb.tile([128, 216], BF16, tag="t2")
        po2 = sb.tile([128, 108], BF16, tag="po2")
        for ci, cs in ((0, 128), (1, 128), (2, 24)):
            nc.scalar.activation(
                t2[0:cs, ci * 72 : ci * 72 + 72], p2it[ci][:, :], AF.Tanh
            )
        for ci, cs in ((0, 128), (1, 128), (2, 24)):
            col = ci * 36
            nc.vector.scalar_tensor_tensor(
                cp2[0:cs, col : col + 36],
                in0=t2[0:cs, 2 * col : 2 * col + 36], scalar=1.0,
                in1=t2[0:cs, 2 * col + 36 : 2 * col + 72],
                op0=OP.add, op1=OP.mult,
            )
            nc.vector.scalar_tensor_tensor(
                po2[0:cs, col : col + 36],
                in0=cp2[0:cs, col : col + 36],
                scalar=aux_v[0:cs, 2 + ci : 3 + ci],
                in1=p2o_regions[ci],
                op0=OP.mult, op1=OP.add,
            )
        tc2 = sb.tile([128, 108], BF16, tag="tc2")
        nc.scalar.activation(tc2, cp2[:, :], AF.Tanh, scale=0.5)
        go2 = sb.tile([128, 108], BF16, tag="go2")
        nc.scalar.activation(go2[0:128, 0:72], po2[0:128, 0:72], AF.Sigmoid)
        nc.scalar.activation(go2[0:24, 72:108], po2[0:24, 72:108], AF.Sigmoid)
        nc.vector.scalar_tensor_tensor(
            x2T[0:128, 0:72], in0=tc2[0:128, 0:72], scalar=0.0,
            in1=go2[0:128, 0:72], op0=OP.max, op1=OP.mult,
        )
        nc.vector.scalar_tensor_tensor(
            x2T[0:24, 72:108], in0=tc2[0:24, 72:108], scalar=0.0,
            in1=go2[0:24, 72:108], op0=OP.max, op1=OP.mult,
        )
        psfc = ps1.tile([N, 36], F32, tag="ps1", name="psfc")
        nc.tensor.matmul(
            psfc, lhsT=x2T[0:128, 0:35], rhs=wb_sb[:, 840:876],
            start=True, stop=False,
        )
        nc.tensor.matmul(
            psfc, lhsT=x2T[0:128, 36:71], rhs=wb_sb[:, 876:912],
            start=False, stop=False,
        )
        nc.tensor.matmul(
            psfc, lhsT=x2T[0:25, 72:107], rhs=wb_sb[0:25, 912:948],
            start=False, stop=True,
        )
        nc.vector.tensor_scalar_max(out_sb[0:N, 0:N], psfc[:, 0:N], 0.0)

    # Output DMA after the tile context, on the warm sync ring.  Fire
    # and forget: its flight hides under the compiler's fixed sem-reset
    # epilogue; ordering comes from the context-exit barrier.  (Moving
    # it inside the context with a second sem update on the relu fails
    # walrus codegen: one sync-update slot per compute instruction.)
    nc.sync.dma_start(out=out[:, :], in_=out_sb[0:N, 0:N]).then_inc(out_sem, 16)

    nc.compile()
    return nc


def pack_inputs(
    adj_matrix, W1, cheb1_b, peep1, b1, W2, cheb2_b, peep2, b2, fc_w, fc_b
) -> dict:
    """Host-side weight packing: gather/concat + bias fold + bf16 cast.

    The I-gate weights (and biases) of both layers and the peephole
    scalars are halved: the kernel computes I via tanh(aI/2)."""
    f = np.float32

    def gate_blk(Wg, bias, scale=1.0):  # [k+1, d] with the bias fold row
        blk = np.concatenate([Wg, bias[None, :]], axis=0) * scale
        return blk.astype(BF)

    adjp = np.zeros((36, 36), dtype=f)
    adjp[0:35, 0:35] = adj_matrix
    adjp[35, 0:35] = 1.0

    wpk_h = np.zeros((36, 456), dtype=BF)
    wpk_h[0:36, 0:36] = adjp.astype(BF)
    wpk_h[0:36, 36:176] = gate_blk(W1[0], cheb1_b[0] + b1[0], 0.5)
    wpk_h[0:36, 176:316] = gate_blk(W1[2], cheb1_b[2] + b1[2])
    wpk_h[0:36, 316:456] = gate_blk(W1[3], cheb1_b[3] + b1[3])

    scales = {0: 0.5, 2: 1.0, 3: 1.0}
    w2k1_h = np.stack(
        [gate_blk(W2[g][128:140], cheb2_b[g] + b2[g], scales[g]) for g in GATES],
        axis=1,
    )  # [13, 3, 280]

    wb_h = np.zeros((128, 958), dtype=BF)
    aux_h = np.zeros((128, 5), dtype=f)
    aux_h[:, 0] = peep1[2][0:128] * 0.5
    aux_h[0:12, 1] = peep1[2][128:140] * 0.5
    aux_h[:, 2] = peep2[2][0:128] * 0.5
    aux_h[:, 3] = peep2[2][128:256] * 0.5
    aux_h[0:24, 4] = peep2[2][256:280] * 0.5
    wb_h[:, 948:958] = np.ascontiguousarray(aux_h).view(BF)
    wb_h[:, 0:280] = (W2[0][0:128] * 0.5).astype(BF)
    wb_h[:, 280:560] = W2[2][0:128].astype(BF)
    wb_h[:, 560:840] = W2[3][0:128].astype(BF)
    fcx = np.concatenate([fc_w, fc_b[None, :]], axis=0)  # [281, 35]
    wb_h[:, 840:875] = fcx[0:128].astype(BF)
    wb_h[:, 876:911] = fcx[128:256].astype(BF)
    wb_h[0:25, 912:947] = fcx[256:281].astype(BF)

    return {
        "wpk": np.ascontiguousarray(wpk_h),
        "w2k1": np.ascontiguousarray(w2k1_h),
        "wb": np.ascontiguousarray(wb_h),
    }


_NC_CACHE: list = []


def kernel(
    adj_matrix,
    W1,
    cheb1_W,
    cheb1_b,
    peep1,
    b1,
    W2,
    cheb2_W,
    cheb2_b,
    peep2,
    b2,
    fc_w,
    fc_b,
) -> np.ndarray:
    from concourse.bass_utils import run_bass_kernel_spmd

    in_map = pack_inputs(
        adj_matrix, W1, cheb1_b, peep1, b1, W2, cheb2_b, peep2, b2, fc_w, fc_b
    )

    if not _NC_CACHE:
        _NC_CACHE.append(build_nc())
    nc = _NC_CACHE[0]

    in_maps = [dict(in_map) for _ in range(N_CORES)]
    try:
        res = run_bass_kernel_spmd(nc, in_maps, core_ids=list(range(N_CORES)))
    except Exception:
        # transient device wedges (NRT_EXEC_*) usually clear on re-run
        res = run_bass_kernel_spmd(nc, in_maps, core_ids=list(range(N_CORES)))
    return np.asarray(res.results[0]["out"], dtype=np.float32)


# revision 47
# speedup vs baseline: 1.0234x; 1.0234x over previous
"""Trainium2 Bass kernel for nn_GCLSTMModel_48868137894020 (v8).

Algebraic reduction (H0 = C0 = 0 kills the cheb convs, the forget gate,
and peep[0/1]); per layer (d = 140 then 280), X = input:
  I = sigmoid(X @ W[0] + cb[0] + b[0])
  T = tanh   (X @ W[2] + cb[2] + b[2])
  C = I * T
  O = sigmoid(X @ W[3] + cb[3] + b[3] + peep[2] * C)
  X' = relu(O * tanh(C)) == max(tanh(C), 0) * O
then out = relu(X'' @ fc_w + fc_b).

Layout: everything transposed (features on partitions, nodes on the
free dim), bf16 matmuls with fp32 PSUM, biases folded via a ones row.

v8 = the v6 compute structure on the v3 two-ring DMA plan (each part
measured best in isolation via neuron-profile traces):
  - sigmoid(x) = (1 + tanh(x/2)) / 2: I-gate weights halved on the
    host, so ONE tanh ACTIVATE per chunk covers gates I and T together
    (ACT-engine serialization was the tail bottleneck).  C' := 2C =
    (1 + tI) * tT is one DVE STT per chunk; tanh(C) = tanh(0.5 * C')
    uses ACT's free scale; peephole scalars are halved on the host.
  - Tile tracks PSUM dependencies at whole-tile granularity, so every
    matmul REGION gets its own psum tile (p1a/p1b/p1o0/... ) - readers
    then wait only for their own producers, and the scheduler runs all
    nine k0 matmuls of the L2 stream before the k1s arrive.
  - DMA: wpk + small w2k1 on the sync HWDGE ring, the big wb pack on
    the scalar ring (see the inline comment for the straggler/table
    trade-offs that pinned this assignment).
  - A warm-up SIGMOID heads the activation sequence: the act-table
    pass keys its table set off the first activation function, and the
    sigmoid set also contains tanh - leading with tanh loads a
    tanh-only set and re-loads mid-chain (v4, +1.3us).
  - Output DMA on the warm sync ring after the tile context; its
    flight hides under the compiler's fixed sem-reset epilogue.

Sharding: problem is tiny (N=35); all 8 cores run the identical program
on replicated inputs (no collectives), output taken from core 0.
"""

import sys

for _p in ("/opt/trn_rl_repo", "/opt/pypackages"):
    if _p not in sys.path:
        sys.path.append(_p)

from contextlib import ExitStack

import numpy as np
import ml_dtypes

import concourse.bacc as bacc
import concourse.bass as bass
import concourse.mybir as mybir
import concourse.tile as tile

F32 = mybir.dt.float32
BF16 = mybir.dt.bfloat16
AF = mybir.ActivationFunctionType
OP = mybir.AluOpType
GATES = (0, 2, 3)  # I, T (cell), O - forget gate (1) is dead
N = 35
D1 = 140
D2 = 280
N_CORES = 8
BF = ml_dtypes.bfloat16

# L1 feature chunks (140 = 128 + 12), L2 chunks (280 = 128 + 256 + 24).
C1 = ((0, 128), (128, 140))
C2 = ((0, 128), (128, 256), (256, 280))


def build_nc() -> bass.Bass:
    nc = bacc.Bacc()

    # wpk: [128, 466] bf16 - adj + ALL layer-1 weights + peepholes:
    #   rows 0:36: cols 0:36 = [adj; ones], 36:176 = [W1[I]/2; bias/2],
    #   176:316 = [W1[T]; bias], 316:456 = [W1[O]; bias]; cols 456:466
    #   rows 0:128 = halved peep scalars as a bit-cast fp32 [128, 5].
    wpk = nc.dram_tensor("wpk", [128, 466], BF16, kind="ExternalInput")
    # W2 k-chunk1 (input features 128:140 + bias row): [13, 3, 280],
    # gate order (I/2, T, O).
    w2k1 = nc.dram_tensor("w2k1", [13, 3, D2], BF16, kind="ExternalInput")
    # wb: [128, 948] bf16 - layer-2 + FC weights (k-chunk0):
    #   cols 0:280 = W2[I]/2, 280:560 = W2[T], 560:840 = W2[O],
    #   840:948 = [fc_w; fc_b] row-chunks as 3x36.
    wb = nc.dram_tensor("wb", [128, 948], BF16, kind="ExternalInput")
    out = nc.dram_tensor("out", [N, N], F32, kind="ExternalOutput")

    out_sb = nc.alloc_sbuf_tensor("out_sbuf", [N, N], F32)
    out_sem = nc.alloc_semaphore("out_dma_sem")

    with ExitStack() as ctx:
        tc = ctx.enter_context(tile.TileContext(nc))
        sb = ctx.enter_context(tc.tile_pool(name="sb", bufs=1))
        ps1 = ctx.enter_context(tc.tile_pool(name="ps1", bufs=3, space="PSUM"))
        ps2 = ctx.enter_context(tc.tile_pool(name="ps2", bufs=5, space="PSUM"))

        # ---- input DMAs: two HWDGE rings issue in parallel ----
        # wpk + small w2k1 on the sync ring, the big wb pack on the
        # scalar ring: queueing wb behind wpk on ONE ring delays wpk's
        # final completion increment by 1-2us (straggler SDMA engine,
        # v5/v6 traces).  The scalar-ring DMA costs a spurious second
        # act-table load, but both loads finish before the first gate
        # activation needs the table (v3/v3d traces).
        wpk_sb = sb.tile([128, 466], BF16, tag="wpk")
        nc.sync.dma_start(out=wpk_sb, in_=wpk[:, :])
        wb_sb = sb.tile([128, 948], BF16, tag="wb")
        nc.scalar.dma_start(out=wb_sb, in_=wb[:, :])
        w2k1_sb = sb.tile([13, 3, D2], BF16, tag="w2k1")
        nc.sync.dma_start(out=w2k1_sb, in_=w2k1[:, :, :])

        adjp = wpk_sb[0:36, 0:36]
        aux_v = wpk_sb[:, 456:466].bitcast(F32)  # [128, 5] peep/2 scalars

        # warm-up sigmoid (no DMA deps): keeps the sigmoid table set
        # (which covers tanh) as the one the table loads converge on,
        # in the DMA shadow.
        warm_src = sb.tile([1, 1], F32, tag="warm_src")
        nc.vector.memset(warm_src[:, :], 0.25)
        warm = sb.tile([1, 1], F32, tag="warm")
        nc.scalar.activation(warm[0:1, 0:1], warm_src[0:1, 0:1], AF.Sigmoid)

        # ones rows for the bias folds + zero-fill for the garbage rows
        # the merged whole-width tanhs read.
        x1T = sb.tile([128, 72], BF16, tag="x1T")
        nc.vector.memset(x1T[0:13, 36:72], 1.0)
        x2T = sb.tile([128, 108], BF16, tag="x2T")
        nc.vector.memset(x2T[0:25, 72:108], 1.0)
        cp1 = sb.tile([128, 72], BF16, tag="cp1")    # C' = 2C, layer 1
        nc.vector.memset(cp1[0:128, 36:72], 0.0)
        cp2 = sb.tile([128, 108], BF16, tag="cp2")   # C' = 2C, layer 2
        nc.vector.memset(cp2[0:128, 72:108], 0.0)

        # ---- layer 1: psum banks grouped for fine-grained waits ----
        # p1a = [Ic0|Tc0]; p1b = [Ic1|Tc1|Oc1] (12 rows); p1o0 = Oc0.
        p1a = ps1.tile([128, 72], F32, tag="ps1", name="p1a")
        p1b = ps1.tile([12, 108], F32, tag="ps1", name="p1b")
        p1o0 = ps1.tile([128, 36], F32, tag="ps1", name="p1o0")
        l1_lhs = (
            lambda a, b: wpk_sb[0:36, 36 + a : 36 + b],   # I/2
            lambda a, b: wpk_sb[0:36, 176 + a : 176 + b],  # T
            lambda a, b: wpk_sb[0:36, 316 + a : 316 + b],  # O
        )
        l1_dsts = (  # (chunk, gate) -> psum region
            (p1a[0:128, 0:36], p1a[0:128, 36:72], p1o0[0:128, 0:36]),
            (p1b[0:12, 0:36], p1b[0:12, 36:72], p1b[0:12, 72:108]),
        )
        for ci, (a, b) in enumerate(C1):
            for g in (0, 1, 2):
                nc.tensor.matmul(
                    l1_dsts[ci][g],
                    lhsT=l1_lhs[g](a, b), rhs=adjp,
                    start=True, stop=True,
                )
        # ACT: t = tanh([aI/2 | aT]) per chunk -> go1 c0 -> tanh(C) -> go1 c1
        t1 = sb.tile([128, 144], BF16, tag="t1")
        nc.scalar.activation(t1[0:128, 0:72], p1a[:, :], AF.Tanh)
        nc.scalar.activation(t1[0:12, 72:144], p1b[0:12, 0:72], AF.Tanh)
        # DVE per chunk: C' = (1+tI)*tT then po = (p/2)*C' + aO.  The
        # chunk-0 x1T STT is emitted BEFORE the chunk-1 peephole STT:
        # the nine k0 matmuls of the L2 stream need only x1T chunk 0,
        # and the v8 trace showed x1Tc0 head-blocked behind po1c1 on
        # the DVE queue for ~0.25us.
        po1 = sb.tile([128, 72], BF16, tag="po1")
        p1o_regions = (p1o0[0:128, 0:36], p1b[0:12, 72:108])
        nc.vector.scalar_tensor_tensor(
            cp1[0:128, 0:36], in0=t1[0:128, 0:36],
            scalar=1.0, in1=t1[0:128, 36:72],
            op0=OP.add, op1=OP.mult,
        )
        nc.vector.scalar_tensor_tensor(
            po1[0:128, 0:36], in0=cp1[0:128, 0:36],
            scalar=aux_v[0:128, 0:1], in1=p1o_regions[0],
            op0=OP.mult, op1=OP.add,
        )
        nc.vector.scalar_tensor_tensor(
            cp1[0:12, 36:72], in0=t1[0:12, 72:108],
            scalar=1.0, in1=t1[0:12, 108:144],
            op0=OP.add, op1=OP.mult,
        )
        go1 = sb.tile([128, 72], BF16, tag="go1")
        nc.scalar.activation(go1[0:128, 0:36], po1[0:128, 0:36], AF.Sigmoid)
        tc1 = sb.tile([128, 72], BF16, tag="tc1")
        nc.scalar.activation(tc1, cp1[:, :], AF.Tanh, scale=0.5)
        # x1T c0 fires as soon as tc1 + go1 c0 land; the chunk-1 chain
        # (po1c1 -> go1c1 -> x1Tc1) trails it on the DVE/ACT queues.
        nc.vector.scalar_tensor_tensor(
            x1T[0:128, 0:36], in0=tc1[0:128, 0:36], scalar=0.0,
            in1=go1[0:128, 0:36], op0=OP.max, op1=OP.mult,
        )
        nc.vector.scalar_tensor_tensor(
            po1[0:12, 36:72], in0=cp1[0:12, 36:72],
            scalar=aux_v[0:12, 1:2], in1=p1o_regions[1],
            op0=OP.mult, op1=OP.add,
        )
        nc.scalar.activation(go1[0:12, 36:72], po1[0:12, 36:72], AF.Sigmoid)
        nc.vector.scalar_tensor_tensor(
            x1T[0:12, 36:72], in0=tc1[0:12, 36:72], scalar=0.0,
            in1=go1[0:12, 36:72], op0=OP.max, op1=OP.mult,
        )

        # ---- layer 2: psum banks grouped for fine-grained waits ----
        # p2a/p2b/p2c = [I|T] per chunk; p2o01 = [Oc0|Oc1]; p2o2 = Oc2.
        p2it = (
            ps2.tile([128, 72], F32, tag="ps2", name="p2a"),
            ps2.tile([128, 72], F32, tag="ps2", name="p2b"),
            ps2.tile([24, 72], F32, tag="ps2", name="p2c"),
        )
        p2o01 = ps2.tile([128, 72], F32, tag="ps2", name="p2o01")
        p2o2 = ps2.tile([24, 36], F32, tag="ps2", name="p2o2")
        p2o_regions = (
            p2o01[0:128, 0:36], p2o01[0:128, 36:72], p2o2[0:24, 0:36]
        )
        w2k0 = (wb_sb[:, 0:280], wb_sb[:, 280:560], wb_sb[:, 560:840])
        for ci, (a, b) in enumerate(C2):
            cs = b - a
            for g, dst in (
                (0, p2it[ci][0:cs, 0:36]),
                (1, p2it[ci][0:cs, 36:72]),
                (2, p2o_regions[ci]),
            ):
                nc.tensor.matmul(
                    dst,
                    lhsT=w2k0[g][:, a:b], rhs=x1T[0:128, 0:36],
                    start=True, stop=False,
                )
                nc.tensor.matmul(
                    dst,
                    lhsT=w2k1_sb[0:13, g, a:b], rhs=x1T[0:13, 36:72],
                    start=False, stop=True,
                )
        t2 = sb.tile([128, 216], BF16, tag="t2")
        po2 = sb.tile([128, 108], BF16, tag="po2")
        for ci, cs in ((0, 128), (1, 128), (2, 24)):
            nc.scalar.activation(
                t2[0:cs, ci * 72 : ci * 72 + 72], p2it[ci][:, :], AF.Tanh
            )
        for ci, cs in ((0, 128), (1, 128), (2, 24)):
            col = ci * 36
            nc.vector.scalar_tensor_tensor(
                cp2[0:cs, col : col + 36],
                in0=t2[0:cs, 2 * col : 2 * col + 36], scalar=1.0,
                in1=t2[0:cs, 2 * col + 36 : 2 * col + 72],
                op0=OP.add, op1=OP.mult,
            )
            nc.vector.scalar_tensor_tensor(
                po2[0:cs, col : col + 36],
                in0=cp2[0:cs, col : col + 36],
                scalar=aux_v[0:cs, 2 + ci : 3 + ci],
                in1=p2o_regions[ci],
                op0=OP.mult, op1=OP.add,
            )
        tc2 = sb.tile([128, 108], BF16, tag="tc2")
        nc.scalar.activation(tc2, cp2[:, :], AF.Tanh, scale=0.5)
        go2 = sb.tile([128, 108], BF16, tag="go2")
        nc.scalar.activation(go2[0:128, 0:72], po2[0:128, 0:72], AF.Sigmoid)
        nc.scalar.activation(go2[0:24, 72:108], po2[0:24, 72:108], AF.Sigmoid)
        nc.vector.scalar_tensor_tensor(
            x2T[0:128, 0:72], in0=tc2[0:128, 0:72], scalar=0.0,
            in1=go2[0:128, 0:72], op0=OP.max, op1=OP.mult,
        )
        nc.vector.scalar_tensor_tensor(
            x2T[0:24, 72:108], in0=tc2[0:24, 72:108], scalar=0.0,
            in1=go2[0:24, 72:108], op0=OP.max, op1=OP.mult,
        )
        psfc = ps1.tile([N, 36], F32, tag="ps1", name="psfc")
        nc.tensor.matmul(
            psfc, lhsT=x2T[0:128, 0:35], rhs=wb_sb[:, 840:876],
            start=True, stop=False,
        )
        nc.tensor.matmul(
            psfc, lhsT=x2T[0:128, 36:71], rhs=wb_sb[:, 876:912],
            start=False, stop=False,
        )
        nc.tensor.matmul(
            psfc, lhsT=x2T[0:25, 72:107], rhs=wb_sb[0:25, 912:948],
            start=False, stop=True,
        )
        nc.vector.tensor_scalar_max(out_sb[0:N, 0:N], psfc[:, 0:N], 0.0)

    # Output DMA after the tile context, on the warm sync ring.  Fire
    # and forget: its flight hides under the compiler's fixed sem-reset
    # epilogue; ordering comes from the context-exit barrier.  (Moving
    # it inside the context with a second sem update on the relu fails
    # walrus codegen: one sync-update slot per compute instruction.)
    nc.sync.dma_start(out=out[:, :], in_=out_sb[0:N, 0:N]).then_inc(out_sem, 16)

    nc.compile()
    return nc


def pack_inputs(
    adj_matrix, W1, cheb1_b, peep1, b1, W2, cheb2_b, peep2, b2, fc_w, fc_b
) -> dict:
    """Host-side weight packing: gather/concat + bias fold + bf16 cast.

    The I-gate weights (and biases) of both layers and the peephole
    scalars are halved: the kernel computes I via tanh(aI/2)."""
    f = np.float32

    def gate_blk(Wg, bias, scale=1.0):  # [k+1, d] with the bias fold row
        blk = np.concatenate([Wg, bias[None, :]], axis=0) * scale
        return blk.astype(BF)

    adjp = np.zeros((36, 36), dtype=f)
    adjp[0:35, 0:35] = adj_matrix
    adjp[35, 0:35] = 1.0

    wpk_h = np.zeros((128, 466), dtype=BF)
    wpk_h[0:36, 0:36] = adjp.astype(BF)
    wpk_h[0:36, 36:176] = gate_blk(W1[0], cheb1_b[0] + b1[0], 0.5)
    wpk_h[0:36, 176:316] = gate_blk(W1[2], cheb1_b[2] + b1[2])
    wpk_h[0:36, 316:456] = gate_blk(W1[3], cheb1_b[3] + b1[3])
    aux_h = np.zeros((128, 5), dtype=f)
    aux_h[:, 0] = peep1[2][0:128] * 0.5
    aux_h[0:12, 1] = peep1[2][128:140] * 0.5
    aux_h[:, 2] = peep2[2][0:128] * 0.5
    aux_h[:, 3] = peep2[2][128:256] * 0.5
    aux_h[0:24, 4] = peep2[2][256:280] * 0.5
    wpk_h[:, 456:466] = np.ascontiguousarray(aux_h).view(BF)

    scales = {0: 0.5, 2: 1.0, 3: 1.0}
    w2k1_h = np.stack(
        [gate_blk(W2[g][128:140], cheb2_b[g] + b2[g], scales[g]) for g in GATES],
        axis=1,
    )  # [13, 3, 280]

    wb_h = np.zeros((128, 948), dtype=BF)
    wb_h[:, 0:280] = (W2[0][0:128] * 0.5).astype(BF)
    wb_h[:, 280:560] = W2[2][0:128].astype(BF)
    wb_h[:, 560:840] = W2[3][0:128].astype(BF)
    fcx = np.concatenate([fc_w, fc_b[None, :]], axis=0)  # [281, 35]
    wb_h[:, 840:875] = fcx[0:128].astype(BF)
    wb_h[:, 876:911] = fcx[128:256].astype(BF)
    wb_h[0:25, 912:947] = fcx[256:281].astype(BF)

    return {
        "wpk": np.ascontiguousarray(wpk_h),
        "w2k1": np.ascontiguousarray(w2k1_h),
        "wb": np.ascontiguousarray(wb_h),
    }


_NC_CACHE: list = []


def kernel(
    adj_matrix,
    W1,
    cheb1_W,
    cheb1_b,
    peep1,
    b1,
    W2,
    cheb2_W,
    cheb2_b,
    peep2,
    b2,
    fc_w,
    fc_b,
) -> np.ndarray:
    from concourse.bass_utils import run_bass_kernel_spmd

    in_map = pack_inputs(
        adj_matrix, W1, cheb1_b, peep1, b1, W2, cheb2_b, peep2, b2, fc_w, fc_b
    )

    if not _NC_CACHE:
        _NC_CACHE.append(build_nc())
    nc = _NC_CACHE[0]

    in_maps = [dict(in_map) for _ in range(N_CORES)]
    try:
        res = run_bass_kernel_spmd(nc, in_maps, core_ids=list(range(N_CORES)))
    except Exception:
        # transient device wedges (NRT_EXEC_*) usually clear on re-run
        res = run_bass_kernel_spmd(nc, in_maps, core_ids=list(range(N_CORES)))
    return np.asarray(res.results[0]["out"], dtype=np.float32)
